# revision 1
# baseline (speedup 1.0000x reference)
"""MobileAttention3D Trainium2 kernel (8-core SPMD).

Sharding: core c -> (b = c//4, hg = c%4) owns batch b and H rows
[8*hg, 8*hg+8).  All conv GEMMs + attention for that slice run locally;
the only cross-core communication is a 32KB AllReduce of partial
attention logits within each batch group {0..3}, {4..7}.

Key layout trick: the reference's torch-style
`o.transpose(0,2,1,3).reshape(B,D,H,W,NH*VD)` scrambles head/spatial
indices as  h' = n*4 + h//8,  w' = (h%8)*4 + w//8,  c2 = (w%8)*64 + vd.
With an H-slice of 8 rows per core this means the attention-value
feature index f = hw_loc*64 + vd factors exactly as w'*512 + c2, so the
attention output lands PSUM-partition-aligned with the proj GEMM's
contraction axis (c2) and no cross-partition shuffle is needed.
"""

import numpy as np
import ml_dtypes

NH, KD, VD, C = 8, 64, 64, 256
B, D, H, W = 2, 32, 32, 32
HS = H // 4            # h rows per core
T = D * HS * W         # 8192 tokens per core
P = 128
NCORES = 8
SCALE = float(VD) ** -0.5

_CACHE = {}


def _build(has_qb, has_kvb, has_pb, sim_mode=False):
    import concourse.bacc as bacc
    import concourse.mybir as mybir
    from concourse import tile

    dt = mybir.dt
    f32, bf16 = dt.float32, dt.bfloat16
    AX = mybir.AxisListType
    AF = mybir.ActivationFunctionType

    nc = bacc.Bacc("TRN2", target_bir_lowering=False, debug=False,
                   enable_asserts=False,
                   num_devices=1 if sim_mode else NCORES)

    x_in = nc.dram_tensor("x", [C, T], bf16, kind="ExternalInput")
    wq_in = nc.dram_tensor("wq", [C, NH * KD], bf16, kind="ExternalInput")
    wkv_in = nc.dram_tensor("wkv", [C, KD + VD], bf16, kind="ExternalInput")
    wp_in = nc.dram_tensor("wp", [NH * VD, C], bf16, kind="ExternalInput")
    idt_in = nc.dram_tensor("idt", [P, P], bf16, kind="ExternalInput")
    qb_in = kvb_in = pb_in = None
    if has_qb:
        qb_in = nc.dram_tensor("qb", [P, NH * KD], bf16, kind="ExternalInput")
    if has_kvb:
        kvb_in = nc.dram_tensor("kvb", [P, KD + VD], bf16, kind="ExternalInput")
    if has_pb:
        # proj bias pre-multiplied by layer_scale, per C channel
        pb_in = nc.dram_tensor("pb", [C, 1], f32, kind="ExternalInput")
    out_t = nc.dram_tensor("out", [C, T], f32, kind="ExternalOutput")

    with tile.TileContext(nc) as tc:
        with tc.tile_pool(name="wpool", bufs=1) as wpool, \
             tc.tile_pool(name="big", bufs=1) as bigpool, \
             tc.tile_pool(name="q2p", bufs=1) as q2pool, \
             tc.tile_pool(name="kvp", bufs=1) as kvpool, \
             tc.tile_pool(name="small", bufs=1) as spool, \
             tc.tile_pool(name="stage", bufs=4) as stpool, \
             tc.tile_pool(name="psum", bufs=4, space="PSUM") as psum, \
             tc.tile_pool(name="dram", bufs=1, space="DRAM") as dram:

            # ---- load weights / constants ----
            wq = wpool.tile([P, 2, NH * KD], bf16)
            wkv = wpool.tile([P, 2, KD + VD], bf16)
            wp = wpool.tile([P, 4, C], bf16)
            idt = wpool.tile([P, P], bf16)
            for ci in range(2):
                nc.sync.dma_start(wkv[:, ci, :], wkv_in[ci * P:(ci + 1) * P, :])
                nc.sync.dma_start(wq[:, ci, :], wq_in[ci * P:(ci + 1) * P, :])
            qb = kvb = pb = None
            if has_qb:
                qb = wpool.tile([P, NH * KD], bf16)
                nc.sync.dma_start(qb[:], qb_in[:])
            if has_kvb:
                kvb = wpool.tile([P, KD + VD], bf16)
                nc.sync.dma_start(kvb[:], kvb_in[:])
            if has_pb:
                pb = wpool.tile([P, 2, 1], f32)
                for ci in range(2):
                    nc.sync.dma_start(pb[:, ci, :], pb_in[ci * P:(ci + 1) * P, :])

            # big slot shared sequentially: x (32KB/p) then oo (64KB/p)
            # load x in column-chunks (both C halves per chunk) so the convs
            # can start as soon as the first token-tiles arrive
            x_sb = bigpool.tile([P, 2, T], bf16, tag="big")
            XCH = 16
            for g in range(XCH):
                lo, hi = g * (T // XCH), (g + 1) * (T // XCH)
                for ci in range(2):
                    eng = nc.sync if ci == 0 else nc.scalar
                    eng.dma_start(x_sb[:, ci, lo:hi],
                                  x_in[ci * P:(ci + 1) * P, lo:hi])
            # weights not needed until later phases
            nc.sync.dma_start(idt[:], idt_in[:])
            for jq in range(4):
                nc.sync.dma_start(wp[:, jq, :], wp_in[jq * P:(jq + 1) * P, :])

            Q2 = q2pool.tile([P, 64 * 512], bf16)     # [p=hw128, (eta, kd, n, dq)]
            ksb = kvpool.tile([P, 64 * KD], bf16)     # [p=hw128, (dk, eta, kd)]
            vsb = kvpool.tile([P, 64 * VD], bf16)     # [p=hw128, (dk, eta, vd)]
            vatt = kvpool.tile([32, 256 * VD], bf16)  # [p=dk, f_loc = hw_loc*64+vd]
            attnT = spool.tile([32, 256], bf16)       # [p=dk, (n,dq)]
            attn = spool.tile([P, 2, 32], bf16)
            l2 = spool.tile([P, 64], f32)
            l3 = spool.tile([P, 64], f32)
            ex = spool.tile([P, 2, 32], f32)
            red = spool.tile([P, 8], f32)

            arin = [dram.tile([P, 32], f32, name=f"arin{mu}")
                    for mu in range(2)]
            arout = [dram.tile([P, 32], f32, name=f"arout{mu}")
                     for mu in range(2)]

            # ---- kv + q convs (style B: tokens on partitions), interleaved
            # per x-chunk so PE consumption tracks x DMA arrival ----
            Q2w = Q2.rearrange("p (e k n dq) -> p dq e n k", e=2, k=KD, n=NH)
            for m in range(16):
                # kv conv for token-tiles [4m, 4m+4)
                ps = psum.tile([P, 512], f32, tag="ps", name=f"pskv{m}")
                for jj in range(4):
                    j = 4 * m + jj
                    for ci in range(2):
                        nc.tensor.matmul(
                            ps[:, jj * P:(jj + 1) * P],
                            x_sb[:, ci, j * P:(j + 1) * P],
                            wkv[:, ci, :],
                            start=(ci == 0), stop=(ci == 1))
                psv = ps.rearrange("p (t c) -> p t c", c=P)
                ks = ksb[:, m * 256:(m + 1) * 256].rearrange("p (t c) -> p t c", c=KD)
                vs = vsb[:, m * 256:(m + 1) * 256].rearrange("p (t c) -> p t c", c=VD)
                if has_kvb:
                    kvbv = kvb.rearrange("p c -> p 1 c")
                    nc.any.tensor_tensor(ks, psv[:, :, 0:KD],
                                         kvbv[:, [0, 0, 0, 0], 0:KD],
                                         op=mybir.AluOpType.add)
                    nc.any.tensor_tensor(vs, psv[:, :, KD:KD + VD],
                                         kvbv[:, [0, 0, 0, 0], KD:KD + VD],
                                         op=mybir.AluOpType.add)
                else:
                    nc.any.tensor_copy(ks, psv[:, :, 0:KD])
                    nc.any.tensor_copy(vs, psv[:, :, KD:KD + VD])
                # q conv for the same token range (dq = 2m, 2m+1)
                for dq in (2 * m, 2 * m + 1):
                    psq = psum.tile([P, 1024], f32, tag="ps", name=f"psq{dq}")
                    for eta in range(2):
                        j = dq * 2 + eta
                        for ci in range(2):
                            nc.tensor.matmul(psq[:, eta * 512:(eta + 1) * 512],
                                             x_sb[:, ci, j * P:(j + 1) * P],
                                             wq[:, ci, :],
                                             start=(ci == 0), stop=(ci == 1))
                    # psum free = (eta,n,kd); Q2 f = eta*16384 + kd*256 + n*32 + dq
                    dst = Q2w[:, dq, :, :, :]
                    src = psq.rearrange("p (e n k) -> p e n k", e=2, n=NH)
                    if has_qb:
                        nc.any.tensor_tensor(
                            dst, src, qb.rearrange("p (n k) -> p 1 n k", n=NH),
                            op=mybir.AluOpType.add)
                    else:
                        nc.any.tensor_copy(dst, src)

            # ---- qk logits: L[(n,dq), dk] partial over local f ----
            # Two independent head-group halves (mu): each gets its own
            # AllReduce so softmax/av of mu=0 overlaps the AR of mu=1.
            Q2v = Q2.rearrange("p (e k m) -> p e k m", e=2, k=KD)
            ksv = ksb.rearrange("p (dk e k) -> p e k dk", e=2, k=KD)
            for mu in range(2):
                psL = psum.tile([P, 32], f32, tag="ps", name=f"psL{mu}")
                for idx, (eta, kd) in enumerate((e, k) for e in range(2)
                                                for k in range(KD)):
                    nc.tensor.matmul(
                        psL[:],
                        Q2v[:, eta, kd, mu * P:(mu + 1) * P],
                        ksv[:, eta, kd, :],
                        start=(idx == 0), stop=(idx == 127))
                nc.any.tensor_copy(l2[:, mu * 32:(mu + 1) * 32], psL[:])
                nc.sync.dma_start(arin[mu][:], l2[:, mu * 32:(mu + 1) * 32])
                if sim_mode:
                    nc.sync.dma_start(arout[mu][:], arin[mu][:])
                else:
                    nc.gpsimd.collective_compute(
                        "AllReduce", mybir.AluOpType.add,
                        replica_groups=[[0, 1, 2, 3], [4, 5, 6, 7]],
                        ins=[arin[mu].opt()], outs=[arout[mu].opt()])
                nc.sync.dma_start(l3[:, mu * 32:(mu + 1) * 32], arout[mu][:])

            # ---- v transpose (fills the AllReduce gap on PE) ----
            # vsb [hw128, (dk,eta,vd)] -> vatt [dk, hw_loc*64+vd]
            vsbv = vsb.rearrange("p (dk e c) -> p e c dk", e=2, c=VD)
            for eta in range(2):
                for vg in range(8):
                    ps = psum.tile([32, 1024], bf16, tag="ps",
                                   name=f"psv{eta}_{vg}")
                    for d_ in range(8):
                        vd = vg * 8 + d_
                        nc.tensor.transpose(ps[0:32, d_ * P:(d_ + 1) * P],
                                            vsbv[:, eta, vd, :], idt[:])
                    src = ps[0:32, :].rearrange("p (d h) -> p d h", h=P)
                    dstv = vatt[0:32, eta * 8192:(eta + 1) * 8192] \
                        .rearrange("p (h v) -> p v h", v=VD)
                    nc.any.tensor_copy(dstv[:, vg * 8:(vg + 1) * 8, :], src)

            oo = bigpool.tile([P, 4, T], bf16, tag="big", name="oo")
            # oo token order f' = w'*256 + n*32 + dq -> proj chunk tch depends
            # only on av outputs for w' in {2tch, 2tch+1} (pipelines av->proj)
            oov = [oo[:, jq, :].rearrange("p (w n dq) -> p w n dq", w=32, n=NH)
                   for jq in range(4)]

            for mu in range(2):
                # ---- softmax over dk (free axis) ----
                sl = l3[:, mu * 32:(mu + 1) * 32]
                mx = red[:, mu * 4 + 0: mu * 4 + 1]
                mxn = red[:, mu * 4 + 1: mu * 4 + 2]
                sm = red[:, mu * 4 + 2: mu * 4 + 3]
                rs = red[:, mu * 4 + 3: mu * 4 + 4]
                nc.vector.reduce_max(mx, sl, axis=AX.X, op=mybir.AluOpType.max)
                nc.scalar.mul(mxn, mx, -SCALE)
                nc.scalar.activation(ex[:, mu, :], sl, AF.Exp,
                                     bias=mxn, scale=SCALE, accum_out=sm)
                nc.vector.reciprocal(rs, sm)
                nc.vector.tensor_scalar_mul(attn[:, mu, :], ex[:, mu, :], rs)

                # ---- attn transpose -> attnT [dk, (n,dq)-half] ----
                pst = psum.tile([32, P], bf16, tag="ps", name=f"psat{mu}")
                nc.tensor.transpose(pst[0:32, :], attn[:, mu, :], idt[:])
                nc.any.tensor_copy(attnT[0:32, mu * P:(mu + 1) * P],
                                   pst[0:32, :])

                # ---- av (this head half) + scramble-drain -> oo ----
                for w4 in range(8):
                    for jq in range(4):
                        ps = psum.tile([P, 512], f32, tag="ps",
                                       name=f"psav{mu}_{w4}_{jq}")
                        for dw in range(4):
                            phi = (4 * w4 + dw) * 4 + jq
                            nc.tensor.matmul(
                                ps[:, dw * P:(dw + 1) * P],
                                vatt[0:32, phi * P:(phi + 1) * P],
                                attnT[0:32, mu * P:(mu + 1) * P],
                                start=True, stop=True)
                        nc.any.tensor_copy(
                            oov[jq][:, 4 * w4:4 * w4 + 4,
                                    mu * 4:(mu + 1) * 4, :],
                            ps.rearrange("p (w n dq) -> p w n dq", w=4, n=4))

            # ---- proj + layer_scale -> out ----
            outv = out_t.rearrange("(ct p) t -> p ct t", p=P)
            for tch in range(16):
                stg = stpool.tile([P, 2, 512], f32, tag="stg",
                                  name=f"stg{tch}")
                for ct in range(2):
                    ps = psum.tile([P, 512], f32, tag="ps", name=f"psp{ct}_{tch}")
                    for jq in range(4):
                        nc.tensor.matmul(ps[:],
                                         wp[:, jq, ct * P:(ct + 1) * P],
                                         oo[:, jq, tch * 512:(tch + 1) * 512],
                                         start=(jq == 0), stop=(jq == 3))
                    # layer_scale is folded into wp on the host
                    if has_pb:
                        nc.any.tensor_scalar_add(stg[:, ct, :], ps[:],
                                                 pb[:, ct, :])
                    else:
                        nc.any.tensor_copy(stg[:, ct, :], ps[:])
                if tch == 15:
                    # split the last store so it pipelines with the drains
                    for ct in range(2):
                        eng = nc.sync if ct == 0 else nc.scalar
                        eng.dma_start(
                            outv[:, ct, tch * 512:(tch + 1) * 512],
                            stg[:, ct, :])
                else:
                    eng = nc.sync if tch % 2 == 0 else nc.scalar
                    eng.dma_start(outv[:, :, tch * 512:(tch + 1) * 512],
                                  stg[:])

    nc.finalize()
    return nc


def _get_nc(has_qb, has_kvb, has_pb):
    key = (has_qb, has_kvb, has_pb)
    if key not in _CACHE:
        _CACHE[key] = _build(*key)
    return _CACHE[key]


def kernel(x, q_w, q_b, kv_w, kv_b, proj_w, proj_b, layer_scale):
    from concourse.bass_utils import run_bass_kernel_spmd
    import os

    x = np.asarray(x, dtype=np.float32)
    q_w = np.asarray(q_w, dtype=np.float32)
    q_b = np.asarray(q_b, dtype=np.float32)
    kv_w = np.asarray(kv_w, dtype=np.float32)
    kv_b = np.asarray(kv_b, dtype=np.float32)
    proj_w = np.asarray(proj_w, dtype=np.float32)
    proj_b = np.asarray(proj_b, dtype=np.float32)
    layer_scale = np.asarray(layer_scale, dtype=np.float32)

    has_qb = bool(np.any(q_b != 0))
    has_kvb = bool(np.any(kv_b != 0))
    has_pb = bool(np.any(proj_b != 0))
    nc = _get_nc(has_qb, has_kvb, has_pb)

    bf = ml_dtypes.bfloat16
    ls_c = layer_scale.reshape(C)                          # [C] f32
    wq = np.ascontiguousarray(q_w.T).astype(bf)            # [C, 512]
    wkv = np.ascontiguousarray(kv_w.T).astype(bf)          # [C, 128]
    # fold layer_scale into proj weights (out = (proj_w @ o) * ls)
    wp = np.ascontiguousarray((proj_w * ls_c[:, None]).T).astype(bf)  # [512, C]
    idt = np.eye(P, dtype=bf)

    shared = {"wq": wq, "wkv": wkv, "wp": wp, "idt": idt}
    if has_qb:
        shared["qb"] = np.broadcast_to(q_b.astype(bf), (P, NH * KD)).copy()
    if has_kvb:
        shared["kvb"] = np.broadcast_to(kv_b.astype(bf), (P, KD + VD)).copy()
    if has_pb:
        shared["pb"] = (proj_b * layer_scale.reshape(-1)).reshape(C, 1) \
            .astype(np.float32)

    in_maps = []
    for c in range(NCORES):
        b, hg = c // 4, c % 4
        xc = np.ascontiguousarray(
            x[b, :, :, hg * HS:(hg + 1) * HS, :].reshape(C, T)).astype(bf)
        in_maps.append({"x": xc, **shared})

    trace = bool(int(os.environ.get("KERNEL_TRACE", "0")))
    res = run_bass_kernel_spmd(nc, in_maps, core_ids=list(range(NCORES)),
                               trace=trace)
    kernel.last_results = res

    out = np.empty((B, C, D, H, W), dtype=np.float32)
    for c in range(NCORES):
        b, hg = c // 4, c % 4
        out[b, :, :, hg::4, :] = res.results[c]["out"] \
            .reshape(C, W, NH, D).transpose(0, 3, 2, 1)
    return out



# revision 15
# speedup vs baseline: 1.4046x; 1.4046x over previous
"""MobileAttention3D Trainium2 kernel (8-core SPMD), v2.

Sharding: core c -> (b = c//4, hg = c%4) owns batch b and H rows
[8*hg, 8*hg+8).  All conv GEMMs + attention for that slice run locally;
the only cross-core communication is a 32KB AllReduce of partial
attention logits within each batch group {0..3}, {4..7}.

v2 changes vs v1 (trace-driven):
  * Q2 free layout -> (dq, eta, kd, n): the q-conv PSUM drain becomes a
    single fully-contiguous 1024-el copy (was 5 ns/el scatter).
  * logits via swapped operands: lhsT = k chunks (32 cols), rhs = Q2
    stream (N=256), 4x column-tiled across PSUM partition strips; strip
    sums + transpose back to [nq, dk] via 4 concurrent fp32 PE
    row-tiles + DVE adds.  Kills 256 128-col LDWEIGHTS.
  * v-transpose done by SBUF->SBUF DMA (128B runs) instead of 128 PE
    transposes + scatter copies; lands directly in the 4-strip layout.
  * attn^T replicated to 4 partition strips with one col-tiled matmul
    quad; attention*V runs 4x row-tiled (contraction dk=32).
  * PSUM evacuation copies alternate between vector and scalar engines.
Output token order is (w', dq, n); host unshard adapts.
"""

import numpy as np
import ml_dtypes

NH, KD, VD, C = 8, 64, 64, 256
B, D, H, W = 2, 32, 32, 32
HS = H // 4            # h rows per core
T = D * HS * W         # 8192 tokens per core
P = 128
NCORES = 8
SCALE = float(VD) ** -0.5

_CACHE = {}


def _build(has_qb, has_kvb, has_pb, sim_mode=False):
    import concourse.bacc as bacc
    import concourse.mybir as mybir
    from concourse import tile

    dt = mybir.dt
    f32, bf16 = dt.float32, dt.bfloat16
    AX = mybir.AxisListType
    AF = mybir.ActivationFunctionType

    nc = bacc.Bacc("TRN2", target_bir_lowering=False, debug=False,
                   enable_asserts=False,
                   num_devices=1 if sim_mode else NCORES)

    x_in = nc.dram_tensor("x", [C, T], bf16, kind="ExternalInput")
    wq_in = nc.dram_tensor("wq", [C, NH * KD], bf16, kind="ExternalInput")
    wkv_in = nc.dram_tensor("wkv", [C, KD + VD], bf16, kind="ExternalInput")
    wp_in = nc.dram_tensor("wp", [NH * VD, C], bf16, kind="ExternalInput")
    idt_in = nc.dram_tensor("idt", [P, P], bf16, kind="ExternalInput")
    idtf_in = nc.dram_tensor("idtf", [P, 32], f32, kind="ExternalInput")
    qb_in = kvb_in = pb_in = None
    if has_qb:
        qb_in = nc.dram_tensor("qb", [P, NH * KD], bf16, kind="ExternalInput")
    if has_kvb:
        kvb_in = nc.dram_tensor("kvb", [P, KD + VD], bf16, kind="ExternalInput")
    if has_pb:
        # proj bias pre-multiplied by layer_scale, per C channel
        pb_in = nc.dram_tensor("pb", [C, 1], f32, kind="ExternalInput")
    out_t = nc.dram_tensor("out", [C, T], f32, kind="ExternalOutput")

    with tile.TileContext(nc) as tc:
        with tc.tile_pool(name="wpool", bufs=1) as wpool, \
             tc.tile_pool(name="big", bufs=1) as bigpool, \
             tc.tile_pool(name="q2p", bufs=1) as q2pool, \
             tc.tile_pool(name="kvp", bufs=1) as kvpool, \
             tc.tile_pool(name="small", bufs=1) as spool, \
             tc.tile_pool(name="stage", bufs=4) as stpool, \
             tc.tile_pool(name="psum", bufs=8, space="PSUM") as psum, \
             tc.tile_pool(name="dram", bufs=1, space="DRAM") as dram:

            # engine rotation for PSUM evacuation copies.
            # (GPSIMD cannot access PSUM, so only vector+scalar rotate.)
            rot_engines = [nc.vector, nc.scalar]
            rot_state = [0]

            def rot_copy(dst, src):
                eng = rot_engines[rot_state[0] % 2]
                rot_state[0] += 1
                if eng is nc.scalar:
                    eng.copy(dst, src)
                else:
                    eng.tensor_copy(dst, src)

            def rot_tt(dst, a, b_, op):
                nc.vector.tensor_tensor(dst, a, b_, op=op)

            # ---- load weights / constants ----
            wq = wpool.tile([P, 2, NH * KD], bf16)
            wkv = wpool.tile([P, 2, KD + VD], bf16)
            wp = wpool.tile([P, 4, C], bf16)
            idt = wpool.tile([P, P], bf16)
            idtf = wpool.tile([P, 32], f32)
            for ci in range(2):
                nc.sync.dma_start(wkv[:, ci, :], wkv_in[ci * P:(ci + 1) * P, :])
                nc.sync.dma_start(wq[:, ci, :], wq_in[ci * P:(ci + 1) * P, :])
            qb = kvb = pb = None
            if has_qb:
                qb = wpool.tile([P, NH * KD], bf16)
                nc.sync.dma_start(qb[:], qb_in[:])
            if has_kvb:
                kvb = wpool.tile([P, KD + VD], bf16)
                nc.sync.dma_start(kvb[:], kvb_in[:])
            if has_pb:
                pb = wpool.tile([P, 2, 1], f32)
                for ci in range(2):
                    nc.sync.dma_start(pb[:, ci, :], pb_in[ci * P:(ci + 1) * P, :])

            # big slot shared sequentially: x (32KB/p) then oo (64KB/p)
            x_sb = bigpool.tile([P, 2, T], bf16, tag="big")
            XCH = 16
            for g in range(XCH):
                lo, hi = g * (T // XCH), (g + 1) * (T // XCH)
                for ci in range(2):
                    eng = nc.sync if ci == 0 else nc.scalar
                    eng.dma_start(x_sb[:, ci, lo:hi],
                                  x_in[ci * P:(ci + 1) * P, lo:hi])
            nc.sync.dma_start(idt[:], idt_in[:])
            nc.sync.dma_start(idtf[:], idtf_in[:])
            for jq in range(4):
                nc.sync.dma_start(wp[:, jq, :], wp_in[jq * P:(jq + 1) * P, :])

            # Q2 [p=hw128, (dq, eta, kd, n)] -- contiguous 1024-el drains
            Q2 = q2pool.tile([P, 32 * 1024], bf16)
            ksb = kvpool.tile([P, 64 * KD], bf16)      # [p=hw128, (dk, eta, kd)]
            vsb = kvpool.tile([P, 64 * VD], bf16)      # [p=hw128, (dk, eta, vd)]
            # vatt4: strip r=[32r..32r+32) holds [dk, (eta, q, b, vd)] for
            # hw128 in [32r, 32r+32);  q = hw128%32 // 2, b = hw128%2
            vatt4 = kvpool.tile([P, 2 * 16 * 2 * VD], bf16)
            attn = spool.tile([P, 2, 32], bf16)
            attnT4 = spool.tile([P, 2, P], bf16)       # attn^T replicated 4 strips
            l2s = spool.tile([P, 256], f32)            # logits strips (dk, nq')
            lsum0 = spool.tile([P, 2, 32], f32)
            lsum1 = spool.tile([P, 2, 32], f32)
            l2 = spool.tile([P, 64], f32)
            l3 = spool.tile([P, 64], f32)
            ex = spool.tile([P, 2, 32], f32)
            red = spool.tile([P, 8], f32)

            arin = [dram.tile([P, 32], f32, name=f"arin{mu}")
                    for mu in range(2)]
            arout = [dram.tile([P, 32], f32, name=f"arout{mu}")
                     for mu in range(2)]

            # ---- kv + q convs (tokens on partitions), interleaved per
            # x-chunk so PE consumption tracks x DMA arrival ----
            for m in range(16):
                ps = psum.tile([P, 512], f32, tag="ps", name=f"pskv{m}")
                for jj in range(4):
                    j = 4 * m + jj
                    for ci in range(2):
                        nc.tensor.matmul(
                            ps[:, jj * P:(jj + 1) * P],
                            x_sb[:, ci, j * P:(j + 1) * P],
                            wkv[:, ci, :],
                            start=(ci == 0), stop=(ci == 1))
                psv = ps.rearrange("p (t c) -> p t c", c=P)
                ks = ksb[:, m * 256:(m + 1) * 256].rearrange("p (t c) -> p t c", c=KD)
                vs = vsb[:, m * 256:(m + 1) * 256].rearrange("p (t c) -> p t c", c=VD)
                if has_kvb:
                    kvbv = kvb.rearrange("p c -> p 1 c")
                    rot_tt(ks, psv[:, :, 0:KD],
                           kvbv[:, [0, 0, 0, 0], 0:KD], mybir.AluOpType.add)
                    rot_tt(vs, psv[:, :, KD:KD + VD],
                           kvbv[:, [0, 0, 0, 0], KD:KD + VD], mybir.AluOpType.add)
                else:
                    rot_copy(ks, psv[:, :, 0:KD])
                    rot_copy(vs, psv[:, :, KD:KD + VD])
                # q conv for the same token range (dq = 2m, 2m+1)
                for dq in (2 * m, 2 * m + 1):
                    for eta in range(2):
                        psq = psum.tile([P, 512], f32, tag="ps",
                                        name=f"psq{dq}_{eta}")
                        j = dq * 2 + eta
                        for ci in range(2):
                            nc.tensor.matmul(psq[:],
                                             x_sb[:, ci, j * P:(j + 1) * P],
                                             wq[:, ci, :],
                                             start=(ci == 0), stop=(ci == 1))
                        # psum free = (kd, n) [wq host col order]; dst is a
                        # contiguous 512-el slice of Q2 (dq, eta, kd, n)
                        dst = Q2[:, dq * 1024 + eta * 512:
                                 dq * 1024 + (eta + 1) * 512]
                        if has_qb:
                            rot_tt(dst.rearrange("p c -> p 1 c"),
                                   psq.rearrange("p c -> p 1 c"),
                                   qb.rearrange("p c -> p 1 c"),
                                   mybir.AluOpType.add)
                        else:
                            rot_copy(dst, psq[:])

            # ---- v "transpose" into vatt4 via SBUF->SBUF DMA ----
            # src strip r: [hw32=(q,b), (dk, eta, vd)]
            # dst strip r: [dk, (eta, q, b, vd)]
            # bounce through DRAM so every SBUF AP is partition-first:
            # DMA1 scatters strip r into vtd[r] as (eta, k, qb, v);
            # DMA2 loads it back contiguously with dk on partitions.
            vtd = [dram.tile([2, 32, 2048], bf16, name=f"vtd{r}")
                   for r in range(4)]
            for r in range(4):
                src1 = vsb[32 * r:32 * (r + 1), :].rearrange(
                    "qb (k e v) -> e qb k v", k=32, e=2, v=VD)
                dst1 = vtd[r].rearrange("e k (qb v) -> e qb k v",
                                        qb=32, v=VD)
                for eta in range(2):
                    eng = nc.sync if (r + eta) % 2 == 0 else nc.scalar
                    eng.dma_start(dst1[eta], src1[eta])
            for r in range(4):
                for eta in range(2):
                    eng = nc.sync if (r + eta) % 2 == 1 else nc.scalar
                    eng.dma_start(
                        vatt4[32 * r:32 * (r + 1),
                              eta * 2048:(eta + 1) * 2048],
                        vtd[r][eta])

            # ---- logits: 4x col-tiled, lhsT = k chunks, rhs = Q2 stream ----
            # psL2 strip c accumulates (eta,kd) idx in [32c, 32c+32)
            Q2v = Q2.rearrange("p (dq e k n) -> p e k dq n",
                               dq=32, e=2, k=KD, n=NH)
            ksv = ksb.rearrange("p (dk e k) -> p e k dk", e=2, k=KD)
            psL2 = psum.tile([P, 256], f32, tag="ps", name="psL2")
            for step in range(32):
                for c_ in range(4):
                    idx = c_ * 32 + step
                    eta, kd = idx // KD, idx % KD
                    nc.tensor.matmul(
                        psL2[32 * c_:32 * (c_ + 1), :],
                        ksv[:, eta, kd, :],
                        Q2v[:, eta, kd, :, :],
                        start=(step == 0), stop=(step == 31),
                        tile_position=(0, 32 * c_), skip_group_check=True)
            nc.vector.tensor_copy(l2s[:], psL2[:])

            # strip sums + transpose back to [nq', dk] via 4 fp32 row-tiles
            for mu in range(2):
                lt = [psum.tile([P, 32], f32, tag="ps", name=f"lt{mu}_{c_}")
                      for c_ in range(4)]
                for c_ in range(4):
                    nc.tensor.matmul(
                        lt[c_][:],
                        l2s[32 * c_:32 * (c_ + 1), mu * P:(mu + 1) * P],
                        idtf[32 * c_:32 * (c_ + 1), :],
                        start=True, stop=True,
                        tile_position=(32 * c_, 0))
                # <=1 PSUM operand per DVE op: stage lt0/lt2 through SBUF
                nc.vector.tensor_copy(lsum0[:, mu, :], lt[0][:])
                nc.scalar.copy(lsum1[:, mu, :], lt[2][:])
                nc.vector.tensor_tensor(lsum0[:, mu, :], lsum0[:, mu, :],
                                        lt[1][:], op=mybir.AluOpType.add)
                nc.vector.tensor_tensor(lsum1[:, mu, :], lsum1[:, mu, :],
                                        lt[3][:], op=mybir.AluOpType.add)
                nc.vector.tensor_tensor(l2[:, mu * 32:(mu + 1) * 32],
                                        lsum0[:, mu, :], lsum1[:, mu, :],
                                        op=mybir.AluOpType.add)
                nc.sync.dma_start(arin[mu][:], l2[:, mu * 32:(mu + 1) * 32])
                if sim_mode:
                    nc.sync.dma_start(arout[mu][:], arin[mu][:])
                else:
                    nc.gpsimd.collective_compute(
                        "AllReduce", mybir.AluOpType.add,
                        replica_groups=[[0, 1, 2, 3], [4, 5, 6, 7]],
                        ins=[arin[mu].opt()], outs=[arout[mu].opt()])
                nc.sync.dma_start(l3[:, mu * 32:(mu + 1) * 32], arout[mu][:])

            oo = bigpool.tile([P, 4, T], bf16, tag="big", name="oo")
            # oo free per jq plane: f' = w'*256 + nq',  nq' = dq*8 + n
            oov = oo.rearrange("p jq (wh wl n) -> p jq wl wh n", wh=8, wl=4)

            def av_group(mu, eta, qh):
                # tile r holds strip r's outputs for all 4 jq (one PSUM
                # bank per row-tile; concurrent row-tiles never share one)
                pr = [psum.tile([P, 512], f32, tag="ps",
                                name=f"psav{mu}_{eta}_{qh}_{r}")
                      for r in range(4)]
                for jq in range(4):
                    q_ = qh * 4 + jq
                    for r in range(4):
                        nc.tensor.matmul(
                            pr[r][:, jq * P:(jq + 1) * P],
                            vatt4[32 * r:32 * (r + 1),
                                  eta * 2048 + q_ * P:eta * 2048 + (q_ + 1) * P],
                            attnT4[32 * r:32 * (r + 1), mu, :],
                            start=True, stop=True,
                            tile_position=(32 * r, 0))
                for r in range(4):
                    # [p, (jq, nq)] -> oo planes jq at w' = eta*16+4r+qh
                    rot_copy(
                        oov[:, :, qh, eta * 4 + r, mu * P:(mu + 1) * P],
                        pr[r].rearrange("p (jq n) -> p jq n", jq=4))

            outv = out_t.rearrange("(ct p) t -> p ct t", p=P)

            def proj_chunk(tch):
                stg = stpool.tile([P, 2, 512], f32, tag="stg",
                                  name=f"stg{tch}")
                for ct in range(2):
                    ps = psum.tile([P, 512], f32, tag="ps",
                                  name=f"psp{ct}_{tch}")
                    for jq in range(4):
                        nc.tensor.matmul(ps[:],
                                         wp[:, jq, ct * P:(ct + 1) * P],
                                         oo[:, jq, tch * 512:(tch + 1) * 512],
                                         start=(jq == 0), stop=(jq == 3))
                    if has_pb:
                        eng = rot_engines[rot_state[0] % 2]
                        rot_state[0] += 1
                        eng.tensor_scalar_add(stg[:, ct, :], ps[:],
                                              pb[:, ct, :])
                    else:
                        rot_copy(stg[:, ct, :], ps[:])
                if tch == 15:
                    for ct in range(2):
                        eng = nc.sync if ct == 0 else nc.scalar
                        eng.dma_start(
                            outv[:, ct, tch * 512:(tch + 1) * 512],
                            stg[:, ct, :])
                else:
                    eng = nc.sync if tch % 2 == 0 else nc.scalar
                    eng.dma_start(outv[:, :, tch * 512:(tch + 1) * 512],
                                  stg[:])

            for mu in range(2):
                # ---- softmax over dk (free axis) ----
                sl = l3[:, mu * 32:(mu + 1) * 32]
                mx = red[:, mu * 4 + 0: mu * 4 + 1]
                mxn = red[:, mu * 4 + 1: mu * 4 + 2]
                sm = red[:, mu * 4 + 2: mu * 4 + 3]
                rs = red[:, mu * 4 + 3: mu * 4 + 4]
                nc.vector.reduce_max(mx, sl, axis=AX.X, op=mybir.AluOpType.max)
                nc.scalar.mul(mxn, mx, -SCALE)
                nc.scalar.activation(ex[:, mu, :], sl, AF.Exp,
                                     bias=mxn, scale=SCALE, accum_out=sm)
                nc.vector.reciprocal(rs, sm)
                nc.vector.tensor_scalar_mul(attn[:, mu, :], ex[:, mu, :], rs)

                # ---- attn^T replicated to 4 strips (col-tiled quad) ----
                psT = psum.tile([P, P], f32, tag="ps", name=f"psat{mu}")
                for r in range(4):
                    nc.tensor.matmul(psT[32 * r:32 * (r + 1), :],
                                     attn[:, mu, :], idt[:, 0:P],
                                     start=True, stop=True,
                                     tile_position=(0, 32 * r))
                nc.vector.tensor_copy(attnT4[:, mu, :], psT[:])

                # ---- attention * V ----
                if mu == 0:
                    for eta in range(2):
                        for qh in range(4):
                            av_group(0, eta, qh)
                else:
                    for eta in range(2):
                        for qhp in range(2):
                            for qh in (2 * qhp, 2 * qhp + 1):
                                av_group(1, eta, qh)
                            for t_ in range(4):
                                proj_chunk(eta * 8 + qhp + 2 * t_)

    nc.finalize()
    return nc


def _get_nc(has_qb, has_kvb, has_pb, sim_mode=False):
    key = (has_qb, has_kvb, has_pb, sim_mode)
    if key not in _CACHE:
        _CACHE[key] = _build(*key)
    return _CACHE[key]


def _host_inputs(q_w, q_b, kv_w, kv_b, proj_w, proj_b, layer_scale,
                 has_qb, has_kvb, has_pb):
    bf = ml_dtypes.bfloat16
    ls_c = layer_scale.reshape(C)                          # [C] f32
    # wq columns reordered to (kd, n) so the q-conv drain is contiguous
    wq = np.ascontiguousarray(
        q_w.reshape(NH, KD, C).transpose(2, 1, 0).reshape(C, NH * KD)
    ).astype(bf)
    wkv = np.ascontiguousarray(kv_w.T).astype(bf)          # [C, 128]
    wp = np.ascontiguousarray((proj_w * ls_c[:, None]).T).astype(bf)
    idt = np.eye(P, dtype=bf)
    idtf = np.tile(np.eye(32, dtype=np.float32), (4, 1))   # [128, 32]

    shared = {"wq": wq, "wkv": wkv, "wp": wp, "idt": idt, "idtf": idtf}
    if has_qb:
        qbr = q_b.reshape(NH, KD).T.reshape(NH * KD)
        shared["qb"] = np.broadcast_to(qbr.astype(bf), (P, NH * KD)).copy()
    if has_kvb:
        shared["kvb"] = np.broadcast_to(kv_b.astype(bf), (P, KD + VD)).copy()
    if has_pb:
        shared["pb"] = (proj_b * layer_scale.reshape(-1)).reshape(C, 1) \
            .astype(np.float32)
    return shared


def kernel(x, q_w, q_b, kv_w, kv_b, proj_w, proj_b, layer_scale):
    from concourse.bass_utils import run_bass_kernel_spmd
    import os

    x = np.asarray(x, dtype=np.float32)
    q_w = np.asarray(q_w, dtype=np.float32)
    q_b = np.asarray(q_b, dtype=np.float32)
    kv_w = np.asarray(kv_w, dtype=np.float32)
    kv_b = np.asarray(kv_b, dtype=np.float32)
    proj_w = np.asarray(proj_w, dtype=np.float32)
    proj_b = np.asarray(proj_b, dtype=np.float32)
    layer_scale = np.asarray(layer_scale, dtype=np.float32)

    has_qb = bool(np.any(q_b != 0))
    has_kvb = bool(np.any(kv_b != 0))
    has_pb = bool(np.any(proj_b != 0))
    nc = _get_nc(has_qb, has_kvb, has_pb)

    bf = ml_dtypes.bfloat16
    shared = _host_inputs(q_w, q_b, kv_w, kv_b, proj_w, proj_b, layer_scale,
                          has_qb, has_kvb, has_pb)

    in_maps = []
    for c in range(NCORES):
        b, hg = c // 4, c % 4
        xc = np.ascontiguousarray(
            x[b, :, :, hg * HS:(hg + 1) * HS, :].reshape(C, T)).astype(bf)
        in_maps.append({"x": xc, **shared})

    trace = bool(int(os.environ.get("KERNEL_TRACE", "0")))
    res = run_bass_kernel_spmd(nc, in_maps, core_ids=list(range(NCORES)),
                               trace=trace)
    kernel.last_results = res

    out = np.empty((B, C, D, H, W), dtype=np.float32)
    for c in range(NCORES):
        b, hg = c // 4, c % 4
        # out token order: t = w'*256 + dq*8 + n
        out[b, :, :, hg::4, :] = res.results[c]["out"] \
            .reshape(C, W, D, NH).transpose(0, 2, 3, 1)
    return out
